# revision 1
# baseline (speedup 1.0000x reference)
"""ABCD spatial module (gnn_message_passing) on 8 TRN2 NeuronCores.

Batch-parallel: core b computes sample b end-to-end (no collectives).
Per core, for each 128-row tile of the [2048, 2048] adjacency space:
  - dyn logits = q@k^T (bf16 PE) + diag(1.6*s_i)@adj (f32 PE, folds the
    0.2*static_adj injection; everything stays at 8x scale)
  - exact top-20 threshold via 16x128-chunk Max8 candidates + 2x
    match_replace rounds on the 128 candidates (DVE, bf16)
  - masked softmax numerators via one scalar_tensor_tensor with fused
    row-sum accumulation; normalization folded into the hybrid blend
  - hybrid matrix assembled in PSUM with 3 diagonal matmuls
    (static / adp / dyn), diagonals carry alpha*w*rownorm scales
  - propagation: one DMA-transpose of the hybrid tile, then 16
    accumulating [128,128]x[128,12] matmuls
"""

import numpy as np

B, T, N, L = 8, 12, 2048, 96
H_DYN, H_ADP, K_TOK = 64, 32, 64
TOPK = 20
TAU = 0.5
STATIC_W = 0.2
ALPHA = 0.2
P = 128
NT = N // P  # 16 row tiles
RSQRT_HADP = 1.0 / np.sqrt(np.float32(H_ADP))

_CACHE = {}
_last_in_maps = None


def _build(repeat: int = 1, lin_ht=False):
    import concourse.bass as bass
    import concourse.tile as tile
    from concourse import bacc, mybir
    from concourse.masks import make_identity

    f32 = mybir.dt.float32
    bf16 = mybir.dt.bfloat16
    Alu = mybir.AluOpType
    Act = mybir.ActivationFunctionType
    AX = mybir.AxisListType

    nc = bacc.Bacc(None, target_bir_lowering=False)

    hist_e = nc.declare_dram_parameter("hist", [L, N], f32, isOutput=False)
    wqt_e = nc.declare_dram_parameter("wqt", [L, H_DYN], f32, isOutput=False)
    wkt_e = nc.declare_dram_parameter("wkt", [L, H_DYN], f32, isOutput=False)
    ukt_e = nc.declare_dram_parameter("ukt", [K_TOK, N], f32, isOutput=False)
    zsrc_e = nc.declare_dram_parameter("zsrc", [K_TOK, H_ADP], f32, isOutput=False)
    zdst_e = nc.declare_dram_parameter("zdst", [K_TOK, H_ADP], f32, isOutput=False)
    adj_e = nc.declare_dram_parameter("adj", [N, N], bf16, isOutput=False)
    xs_e = nc.declare_dram_parameter("xs", [P, NT, T], f32, isOutput=False)
    wv_e = nc.declare_dram_parameter("wv", [1, 3], f32, isOutput=False)
    out_e = nc.declare_dram_parameter("out", [NT, P, T], f32, isOutput=True)

    with tile.TileContext(nc) as tc:
        from contextlib import ExitStack

        ctx = ExitStack()
        with ctx:
            const = ctx.enter_context(tc.tile_pool(name="const", bufs=1))
            up = ctx.enter_context(tc.tile_pool(name="up", bufs=3))
            work = ctx.enter_context(tc.tile_pool(name="work", bufs=3))
            small = ctx.enter_context(tc.tile_pool(name="small", bufs=4))
            pl = ctx.enter_context(tc.tile_pool(name="pl", bufs=2, space="PSUM"))

            # ---------- constants / params ----------
            ident_f = const.tile([P, P], f32)
            make_identity(nc, ident_f)
            ident_b = const.tile([P, P], bf16)
            nc.vector.tensor_copy(ident_b, ident_f)

            hist_sb = const.tile([L, N], f32)
            nc.sync.dma_start(out=hist_sb, in_=hist_e[:, :])
            wq_sb = const.tile([L, H_DYN], f32)
            nc.sync.dma_start(out=wq_sb, in_=wqt_e[:, :])
            wk_sb = const.tile([L, H_DYN], f32)
            nc.sync.dma_start(out=wk_sb, in_=wkt_e[:, :])
            ukt_sb = const.tile([K_TOK, N], f32)
            nc.sync.dma_start(out=ukt_sb, in_=ukt_e[:, :])
            zs_sb = const.tile([K_TOK, H_ADP], f32)
            nc.sync.dma_start(out=zs_sb, in_=zsrc_e[:, :])
            zd_sb = const.tile([K_TOK, H_ADP], f32)
            nc.sync.dma_start(out=zd_sb, in_=zdst_e[:, :])
            xs_f = const.tile([P, NT, T], f32)
            nc.sync.dma_start(out=xs_f, in_=xs_e[:, :, :])
            xs_b = const.tile([P, NT, T], bf16)
            nc.vector.tensor_copy(xs_b, xs_f)
            # blend weights w = softmax(hybrid_logits), broadcast to all
            # partitions so per-partition scalar APs can read them.
            wv_sb = const.tile([P, 3], f32)
            nc.sync.dma_start(
                out=wv_sb,
                in_=bass.AP(
                    tensor=wv_e[:, :].tensor,
                    offset=wv_e[:, :].offset,
                    ap=[[0, P]] + list(wv_e[:, :].ap[1:]),
                ),
            )

            # ---------- upfront: q, k, src, dst (normalized, transposed) ----
            def build_qkT(name, lhs_sb, rhs_sb, hdim, contract):
                """rows = l2norm(lhs^T @ rhs) computed 128 rows at a time,
                then one DMA transpose to [128(pad), NT, P] bf16. The HW xbar
                maps out[p', c, f] = in[f, c*128 + p'], so the head dim is
                padded to 128 in the pre-transpose free layout."""
                all_b = const.tile([P, NT, P], bf16, name=f"all_{name}")
                nc.vector.memset(all_b, 0.0)
                for c in range(NT):
                    ps = pl.tile([P, hdim], f32, tag="big")
                    nc.tensor.matmul(
                        ps,
                        lhs_sb[:, c * P : (c + 1) * P],
                        rhs_sb,
                        start=True,
                        stop=True,
                    )
                    scr = up.tile([P, hdim], bf16)
                    ssq = small.tile([P, 1], f32)
                    nc.scalar.activation(scr, ps, Act.Square, accum_out=ssq)
                    nrm = small.tile([P, 1], f32)
                    nc.scalar.activation(nrm, ssq, Act.Sqrt)
                    nc.vector.tensor_scalar_max(nrm, nrm, 1e-12)
                    rq = small.tile([P, 1], f32)
                    nc.vector.reciprocal(rq, nrm)
                    nc.scalar.activation(
                        all_b[:, c, 0:hdim], ps, Act.Copy, scale=rq
                    )
                tr = const.tile([P, NT, P], bf16, name=f"tr_{name}")
                nc.sync.dma_start_transpose(tr, all_b[:, :, :].rearrange("p c h -> p (c h)"))
                return tr

            qT = build_qkT("q", hist_sb, wq_sb, H_DYN, L)
            kT = build_qkT("k", hist_sb, wk_sb, H_DYN, L)
            srcT = build_qkT("src", ukt_sb, zs_sb, H_ADP, K_TOK)
            dstT = build_qkT("dst", ukt_sb, zd_sb, H_ADP, K_TOK)

            NEG = -1.0e30

            # ---------- main loop over row tiles ----------
            # Software-pipelined: stage1(r) = logits + top-k + masked-e
            # (PSUM ring slots: qk, ap); stage2(r) = hybrid assembly +
            # transpose + propagation (slots: H, acc). Emitting stage2(r-1)
            # after stage1(r) gives the scheduler a 2-tile window, so tile
            # r+1's DMA/matmul/top-k overlaps tile r's propagation tail.
            def topk_thr(x_bf):
                cand = small.tile([P, NT * 8], bf16, name="cand")
                for c in range(NT):
                    nc.vector.max(
                        out=cand[:, c * 8 : (c + 1) * 8],
                        in_=x_bf[:, c * P : (c + 1) * P],
                    )
                m1 = small.tile([P, 8], bf16, name="m1")
                nc.vector.max(out=m1, in_=cand)
                c2 = small.tile([P, NT * 8], bf16, name="c2")
                nc.vector.match_replace(
                    out=c2, in_to_replace=m1, in_values=cand, imm_value=NEG
                )
                m2 = small.tile([P, 8], bf16, name="m2")
                nc.vector.max(out=m2, in_=c2)
                c3 = small.tile([P, NT * 8], bf16, name="c3")
                nc.vector.match_replace(
                    out=c3, in_to_replace=m2, in_values=c2, imm_value=NEG
                )
                m3 = small.tile([P, 8], bf16, name="m3")
                nc.vector.max(out=m3, in_=c3)
                return m1, m3[:, (TOPK - 17) : (TOPK - 16)]

            def stage1(r):
                st = {}
                adj_r = work.tile([P, N], bf16, name="adj_r")
                nc.gpsimd.dma_start(out=adj_r, in_=adj_e[r * P : (r + 1) * P, :])
                st["adj"] = adj_r

                # s_i = 1/max(rowsum, 1e-6); rowsum on ScalarE fused accum
                rs = small.tile([P, 1], f32, name="rs")
                rs_scr = work.tile([P, N], bf16, name="rs_scr")
                nc.scalar.activation(rs_scr, adj_r, Act.Copy, accum_out=rs)
                nc.vector.tensor_scalar_max(rs, rs, 1e-6)
                s_i = small.tile([P, 1], f32, name="s_i")
                nc.vector.reciprocal(s_i, rs)
                st["s_i"] = s_i

                # dyn logits (x8 scale): q@k^T + 1.6*s_i*adj
                diag16 = small.tile([P, P], bf16, name="diag16")
                nc.vector.tensor_scalar(
                    diag16, ident_f, s_i, 8.0 * STATIC_W, Alu.mult, Alu.mult
                )
                qk_ps = pl.tile([P, N], f32, tag="big", name="qk_ps")
                kT_f = kT[0:H_DYN, :, :].rearrange("h c p -> h (c p)")
                for j in range(4):
                    js = slice(j * 512, (j + 1) * 512)
                    nc.tensor.matmul(
                        qk_ps[:, js], qT[0:H_DYN, r, :], kT_f[:, js],
                        start=True, stop=False,
                    )
                    nc.tensor.matmul(
                        qk_ps[:, js], diag16, adj_r[:, js],
                        start=False, stop=True,
                    )
                dl = work.tile([P, N], bf16, name="dl")
                nc.scalar.activation(dl, qk_ps, Act.Copy)

                # adp logits (x sqrt(32) scale)
                ap_ps = pl.tile([P, N], f32, tag="big", name="ap_ps")
                dstT_f = dstT[0:H_ADP, :, :].rearrange("h c p -> h (c p)")
                for j in range(4):
                    js = slice(j * 512, (j + 1) * 512)
                    nc.tensor.matmul(
                        ap_ps[:, js], srcT[0:H_ADP, r, :], dstT_f[:, js],
                        start=True, stop=True,
                    )
                al = work.tile([P, N], bf16, name="al")
                nc.scalar.activation(al, ap_ps, Act.Copy)

                # dyn top-20 -> masked-e + row sums
                dm1, dthr = topk_thr(dl)
                bias_d = small.tile([P, 1], f32, name="bias_d")
                nc.gpsimd.tensor_scalar(bias_d, dm1[:, 0:1], -0.25, None,
                                        Alu.mult)
                e_d = work.tile([P, N], bf16, name="e_d")
                nc.scalar.activation(e_d, dl, Act.Exp, bias=bias_d, scale=0.25)
                de = work.tile([P, N], bf16, name="de")
                dsum = small.tile([P, 1], f32, name="dsum")
                nc.vector.scalar_tensor_tensor(
                    out=de, in0=dl, scalar=dthr, in1=e_d,
                    op0=Alu.is_ge, op1=Alu.mult, accum_out=dsum,
                )
                st["de"], st["dsum"] = de, dsum

                # adp top-20 -> masked-e + row sums
                am1, athr = topk_thr(al)
                sc_a = float(2.0 * RSQRT_HADP)
                bias_a = small.tile([P, 1], f32, name="bias_a")
                nc.gpsimd.tensor_scalar(bias_a, am1[:, 0:1], -sc_a, None,
                                        Alu.mult)
                e_a = work.tile([P, N], bf16, name="e_a")
                nc.scalar.activation(e_a, al, Act.Exp, bias=bias_a, scale=sc_a)
                ae = work.tile([P, N], bf16, name="ae")
                asum = small.tile([P, 1], f32, name="asum")
                nc.vector.scalar_tensor_tensor(
                    out=ae, in0=al, scalar=athr, in1=e_a,
                    op0=Alu.is_ge, op1=Alu.mult, accum_out=asum,
                )
                st["ae"], st["asum"] = ae, asum
                return st

            def stage2(r, st):
                # H = alpha*(w0*s_i*adj + w1*ae/asum + w2*de/dsum) via
                # diagonal matmuls accumulated in PSUM
                adj_r, s_i = st["adj"], st["s_i"]
                de, dsum, ae, asum = st["de"], st["dsum"], st["ae"], st["asum"]
                diagS = small.tile([P, P], bf16, name="diagS")
                nc.vector.scalar_tensor_tensor(
                    out=diagS, in0=ident_f, scalar=s_i, in1=ident_f,
                    op0=Alu.mult, op1=Alu.bypass,
                )
                nc.vector.tensor_scalar(
                    diagS, diagS, wv_sb[:, 0:1], ALPHA, Alu.mult, Alu.mult
                )
                rd = small.tile([P, 1], f32, name="rd")
                nc.vector.reciprocal(rd, dsum)
                diagD = small.tile([P, P], bf16, name="diagD")
                nc.vector.tensor_scalar(
                    diagD, ident_f, rd, wv_sb[:, 2:3], Alu.mult, Alu.mult
                )
                ra = small.tile([P, 1], f32, name="ra")
                nc.vector.reciprocal(ra, asum)
                diagA = small.tile([P, P], bf16, name="diagA")
                nc.vector.tensor_scalar(
                    diagA, ident_f, ra, wv_sb[:, 1:2], Alu.mult, Alu.mult
                )
                h_ps = pl.tile([P, N], f32, tag="big", name="h_ps")
                for j in range(4):
                    js = slice(j * 512, (j + 1) * 512)
                    nc.tensor.matmul(h_ps[:, js], diagS, adj_r[:, js],
                                     start=True, stop=False)
                    nc.tensor.matmul(h_ps[:, js], diagD, de[:, js],
                                     start=False, stop=False)
                    nc.tensor.matmul(h_ps[:, js], diagA, ae[:, js],
                                     start=False, stop=True)
                h_sb = work.tile([P, N], bf16, name="h_sb")
                nc.scalar.activation(h_sb, h_ps, Act.Copy, scale=ALPHA)

                # propagation: one DMA block-transpose + 16 accum matmuls
                hT = work.tile([P, NT, P], bf16, name="hT")
                if lin_ht:
                    nc.sync.dma_start(
                        out=hT[:, :, :].rearrange("a c b -> a (c b)"), in_=h_sb
                    )
                else:
                    nc.sync.dma_start_transpose(hT, h_sb)
                acc = pl.tile([P, T], f32, tag="big", name="acc")
                for c in range(NT):
                    nc.tensor.matmul(
                        acc, hT[:, c, :], xs_b[:, c, :],
                        start=(c == 0), stop=(c == NT - 1),
                    )
                out_sb = small.tile([P, T], f32, name="out_sb")
                nc.vector.tensor_add(out_sb, acc, xs_f[:, r, :])
                nc.gpsimd.dma_start(out=out_e[r], in_=out_sb)

            for _rep in range(repeat):
                prev = None
                for r in range(NT):
                    st = stage1(r)
                    if prev is not None:
                        stage2(r - 1, prev)
                    prev = st
                stage2(NT - 1, prev)

    nc.compile()
    return nc


def _get_nc():
    if "nc" not in _CACHE:
        _CACHE["nc"] = _build()
    return _CACHE["nc"]


def kernel(output, history_flow, Wq, Wk, Z_src, Z_dst, U_k, hybrid_logits,
           adj_mx):
    from concourse.bass_utils import run_bass_kernel_spmd

    output = np.asarray(output, np.float32)
    history_flow = np.asarray(history_flow, np.float32)
    import ml_dtypes

    adj_bf = np.ascontiguousarray(
        np.asarray(adj_mx, np.float32).astype(ml_dtypes.bfloat16)
    )
    wqt = np.ascontiguousarray(np.asarray(Wq, np.float32).T)
    wkt = np.ascontiguousarray(np.asarray(Wk, np.float32).T)
    ukt = np.ascontiguousarray(np.asarray(U_k, np.float32).T)
    zsrc = np.ascontiguousarray(np.asarray(Z_src, np.float32))
    zdst = np.ascontiguousarray(np.asarray(Z_dst, np.float32))
    hl = np.asarray(hybrid_logits, np.float64)
    w = np.exp(hl - hl.max())
    w = (w / w.sum()).astype(np.float32).reshape(1, 3)

    nc = _get_nc()
    in_maps = []
    for b in range(B):
        xb = output[b, :, :, 0]  # [T, N]
        xs = np.ascontiguousarray(
            xb.T.reshape(NT, P, T).transpose(1, 0, 2)
        )  # [P, NT, T]
        in_maps.append(
            {
                "hist": np.ascontiguousarray(history_flow[b]),
                "wqt": wqt,
                "wkt": wkt,
                "ukt": ukt,
                "zsrc": zsrc,
                "zdst": zdst,
                "adj": adj_bf,
                "xs": xs,
                "wv": w,
            }
        )

    global _last_in_maps
    _last_in_maps = in_maps
    res = run_bass_kernel_spmd(nc, in_maps, core_ids=list(range(B)))
    out = np.empty((B, T, N, 1), np.float32)
    for b in range(B):
        ob = res.results[b]["out"].reshape(N, T)  # [NT*P, T]
        out[b, :, :, 0] = ob.T
    return out

